# revision 21
# baseline (speedup 1.0000x reference)
# BinsCombinerLayer Trainium2 kernel.
#
#   out[b] = (1/NUM_BINS) * sum_{n,s} inputs[b,n,s] * centroids[n,s]
#
# Pure data parallel over 8 NeuronCores: each core takes B/8 = 4096 examples.
# The dot product runs on the PE array in bf16 (the 2e-2 tolerance leaves
# orders of magnitude of slack), which halves the HBM traffic vs f32 --
# the binding roofline for this kernel -- and frees the DVE entirely.
#
# Host-side prep per core: x slice [4096, 2048] f32 is cast to bf16 and
# transposed to xT [2048, 4096] (d-major) so the PE can contract over the
# partition axis: for each 128-row d-chunk k, matmul(psum[1, N], lhsT =
# cbT[:, k] [128, 1], rhs = xT_k [128, N]) accumulates the per-example
# partial dots over the 16 chunks in PSUM.  Centroids are pre-scaled by
# 1/NUM_BINS and transposed to [128, 16] on host (tiny).
import numpy as np

import concourse.bacc as bacc
import concourse.mybir as mybir
import concourse.tile as tile
from concourse.bass_utils import run_bass_kernel_spmd

N_CORES = 8
B, NUM_BINS, BIN_SIZE = 32768, 16, 128
D = NUM_BINS * BIN_SIZE      # 2048 f32 per example
P = 128                      # SBUF partitions
BC = B // N_CORES            # 4096 examples per core
K = D // P                   # 16 d-chunks of 128
F32 = mybir.dt.float32
BF16 = mybir.dt.bfloat16

_CACHED = None


def _build_program(repeat=1, qw=1024, nblk=512, bufs=12, dual_q=False,
                   out_q="sync", tilemajor=True, mm_width=None):
    """Per pass: 4 quarters of qw examples; per quarter, 16 chunk DMAs
    (one per 128-row d-chunk) feed qw/nblk PSUM accumulation groups.
    Matmuls run k-outer within the quarter so chunk k's matmuls fire as its
    DMA lands and the post-last-DMA tail is just the final k's matmuls plus
    the PSUM drains; drain copies alternate ACT/DVE."""
    nc = bacc.Bacc("TRN2", target_bir_lowering=False, debug=False)
    nq = BC // qw
    if tilemajor:
        # tile-major DRAM layout: each (quarter, chunk) DMA reads one fully
        # contiguous 128*qw*2B extent (better HBM row locality)
        x = nc.dram_tensor("x", [nq * K * P, qw], BF16, kind="ExternalInput").ap()
    else:
        x = nc.dram_tensor("x", [D, BC], BF16, kind="ExternalInput").ap()
    cb = nc.dram_tensor("cb", [P, K], BF16, kind="ExternalInput").ap()
    out = nc.dram_tensor("out", [1, BC], F32, kind="ExternalOutput").ap()
    NB = qw // nblk
    with tile.TileContext(nc) as tc:
        with (
            tc.tile_pool(name="xin", bufs=bufs) as xpool,
            tc.tile_pool(name="misc", bufs=1) as misc,
            tc.tile_pool(
                name="ps", bufs=min(8, max(4, 2 * NB)), space="PSUM"
            ) as pspool,
        ):
            cbt = misc.tile([P, K], BF16)
            # scalar (ACT) HWDGE queue: runs parallel to the x stream on sync
            nc.scalar.dma_start(out=cbt[:], in_=cb[:])
            collect = misc.tile([1, BC], F32)

            for _ in range(repeat):
                for q in range(nq):
                    xts = []
                    for k in range(K):
                        xt = xpool.tile([P, qw], BF16, tag="xt")
                        eng = nc.scalar if (dual_q and k % 2) else nc.sync
                        if tilemajor:
                            r = (q * K + k) * P
                            src = x[r : r + P, :]
                        else:
                            src = x[k * P : (k + 1) * P, q * qw : (q + 1) * qw]
                        eng.dma_start(out=xt[:], in_=src)
                        xts.append(xt)
                    pss = [
                        pspool.tile([1, nblk], F32, tag="ps", name=f"ps_{b}")
                        for b in range(NB)
                    ]
                    mw = mm_width or nblk
                    for k in range(K):
                        for blk in range(NB):
                            lo = blk * nblk
                            nc.tensor.matmul(
                                pss[blk][:, :mw],
                                cbt[:, k : k + 1],
                                xts[k][:, lo : lo + mw],
                                start=(k == 0),
                                stop=(k == K - 1),
                            )
                    for blk in range(NB):
                        seg = collect[
                            :, q * qw + blk * nblk : q * qw + (blk + 1) * nblk
                        ]
                        if blk % 2:
                            nc.vector.tensor_copy(seg, pss[blk][:])
                        else:
                            nc.scalar.copy(seg, pss[blk][:])

                getattr(nc, out_q).dma_start(out=out[:], in_=collect[:])

    nc.compile()
    return nc


def _get_program():
    global _CACHED
    if _CACHED is None:
        _CACHED = _build_program_v6()
    return _CACHED


# ---- v6: PE + DVE split -----------------------------------------------------
# The DMA stream sustains ~740 GB/s/core but the PE alone caps the pass at
# ~27.5 us (1 bf16 column/cycle).  Offload the last PE_E..BC examples to the
# (otherwise idle) DVE as 1x scalar_tensor_tensor dots in example-major
# layout, balancing PE ~21 us / DVE ~18 us under the ~23 us DMA floor.
PE_E = 3072            # examples on the PE (3 tile-major quarters)
DVE_E = BC - PE_E      # 1024 examples on the DVE
TD = DVE_E // P        # 8 example-slots per partition


def _build_program_v6(repeat=1, qw=1024, nblk=512, bufs=12):
    nc = bacc.Bacc("TRN2", target_bir_lowering=False, debug=False)
    nq = PE_E // qw
    x = nc.dram_tensor("x", [nq * K * P, qw], BF16, kind="ExternalInput").ap()
    xe = nc.dram_tensor("xe", [(TD // 2) * P, 2 * D], BF16, kind="ExternalInput").ap()
    cb = nc.dram_tensor("cb", [P, K], BF16, kind="ExternalInput").ap()
    cbb = nc.dram_tensor("cbb", [P, D], BF16, kind="ExternalInput").ap()
    out = nc.dram_tensor("out", [1, PE_E], F32, kind="ExternalOutput").ap()
    out2 = nc.dram_tensor("out2", [P, TD], F32, kind="ExternalOutput").ap()
    NB = qw // nblk
    with tile.TileContext(nc) as tc:
        with (
            tc.tile_pool(name="xin", bufs=bufs) as xpool,
            tc.tile_pool(name="xein", bufs=3) as xepool,
            tc.tile_pool(name="misc", bufs=1) as misc,
            tc.tile_pool(name="ps", bufs=6, space="PSUM") as pspool,
            tc.tile_pool(name="psd", bufs=1, space="PSUM") as psdpool,
        ):
            cbt = misc.tile([P, K], BF16)
            nc.scalar.dma_start(out=cbt[:], in_=cb[:])
            cbbt = misc.tile([P, D], BF16)
            nc.scalar.dma_start(out=cbbt[:], in_=cbb[:])
            collect = misc.tile([1, PE_E], F32)
            colle = misc.tile([P, TD], F32)
            # STT elementwise result is discarded into a stride-0 PSUM dummy
            dummy = psdpool.tile([P, 1], F32)

            for _ in range(repeat):
                # DVE stream: TD//2 example-pair tiles, extent-contiguous
                xets = []
                for j in range(TD // 2):
                    xet = xepool.tile([P, 2 * D], BF16, tag="xe", name=f"xe_{j}")
                    nc.sync.dma_start(out=xet[:], in_=xe[j * P : (j + 1) * P, :])
                    xets.append(xet)
                # PE stream: tile-major quarters
                for q in range(nq):
                    xts = []
                    for k in range(K):
                        xt = xpool.tile([P, qw], BF16, tag="xt")
                        r = (q * K + k) * P
                        nc.sync.dma_start(out=xt[:], in_=x[r : r + P, :])
                        xts.append(xt)
                    if q == 0:
                        # DVE dots run concurrently with the PE quarters
                        for j in range(TD // 2):
                            for h in range(2):
                                t = 2 * j + h
                                nc.vector.scalar_tensor_tensor(
                                    out=dummy.broadcast_to((P, D)),
                                    in0=xets[j][:, h * D : (h + 1) * D],
                                    scalar=1.0,
                                    in1=cbbt[:],
                                    op0=mybir.AluOpType.mult,
                                    op1=mybir.AluOpType.mult,
                                    accum_out=colle[:, t : t + 1],
                                )
                    pss = [
                        pspool.tile([1, nblk], F32, tag="ps", name=f"ps_{b}")
                        for b in range(NB)
                    ]
                    for k in range(K):
                        for blk in range(NB):
                            lo = blk * nblk
                            nc.tensor.matmul(
                                pss[blk][:],
                                cbt[:, k : k + 1],
                                xts[k][:, lo : lo + nblk],
                                start=(k == 0),
                                stop=(k == K - 1),
                            )
                    for blk in range(NB):
                        seg = collect[
                            :, q * qw + blk * nblk : q * qw + (blk + 1) * nblk
                        ]
                        if blk % 2:
                            nc.vector.tensor_copy(seg, pss[blk][:])
                        else:
                            nc.scalar.copy(seg, pss[blk][:])

                nc.scalar.dma_start(out=out2[:], in_=colle[:])
                nc.sync.dma_start(out=out[:], in_=collect[:])

    nc.compile()
    return nc


def _prep_inputs_v6(inputs, centroids, qw=1024):
    import ml_dtypes

    bf16 = ml_dtypes.bfloat16
    x = np.asarray(inputs, dtype=np.float32).reshape(N_CORES, BC, D)
    xbf = x.astype(bf16)
    nq = PE_E // qw
    # PE part: tile-major d-major layout of the first PE_E examples
    xT = np.ascontiguousarray(
        xbf[:, :PE_E, :].transpose(0, 2, 1)
    )  # [cores, D, PE_E]
    xT = np.ascontiguousarray(
        xT.reshape(N_CORES, K, P, nq, qw).transpose(0, 3, 1, 2, 4)
    ).reshape(N_CORES, nq * K * P, qw)
    # DVE part: example-major, b = PE_E + TD*p + t, blocked into
    # extent-contiguous [P, 2*D] pair-tiles
    xd = xbf[:, PE_E:, :].reshape(N_CORES, P, TD, D)
    xe = np.ascontiguousarray(
        xd.reshape(N_CORES, P, TD // 2, 2 * D).transpose(0, 2, 1, 3)
    ).reshape(N_CORES, (TD // 2) * P, 2 * D)
    c = np.asarray(centroids, dtype=np.float32).reshape(D) / NUM_BINS
    cb16 = c.astype(bf16)
    cbT = np.ascontiguousarray(cb16.reshape(K, P).T)
    cbb = np.ascontiguousarray(np.broadcast_to(cb16, (P, D)))
    return xT, xe, cbT, cbb


def _prep_inputs(inputs, centroids, qw=1024, tilemajor=True):
    import ml_dtypes

    bf16 = ml_dtypes.bfloat16
    x = np.asarray(inputs, dtype=np.float32).reshape(N_CORES, BC, D)
    # cast + transpose to per-core [D, BC] bf16 (d-major, examples contiguous)
    xT = np.ascontiguousarray(x.transpose(0, 2, 1)).astype(bf16)
    if tilemajor:
        nq = BC // qw
        # [cores, D, BC] -> [cores, nq*K*P, qw] with (q, k) tiles contiguous
        xT = np.ascontiguousarray(
            xT.reshape(N_CORES, K, P, nq, qw).transpose(0, 3, 1, 2, 4)
        ).reshape(N_CORES, nq * K * P, qw)
    c = np.asarray(centroids, dtype=np.float32).reshape(D) / NUM_BINS
    # cbT[p, k] = c[k*128 + p]
    cbT = np.ascontiguousarray(c.astype(bf16).reshape(K, P).T)
    return xT, cbT


def run(inputs, centroids, **spmd_kwargs):
    """Run the kernel; returns (full_output, BassKernelResults)."""
    global _CACHED
    if _CACHED is None:
        _CACHED = _build_program_v6()
    nc = _CACHED
    xT, xe, cbT, cbb = _prep_inputs_v6(inputs, centroids)
    in_maps = [
        {"x": xT[i], "xe": xe[i], "cb": cbT, "cbb": cbb}
        for i in range(N_CORES)
    ]
    try:
        res = run_bass_kernel_spmd(
            nc, in_maps, list(range(N_CORES)), **spmd_kwargs
        )
    except Exception:
        # transient NRT_EXEC_UNIT_UNRECOVERABLE wedges recover on retry
        res = run_bass_kernel_spmd(
            nc, in_maps, list(range(N_CORES)), **spmd_kwargs
        )
    full = np.concatenate(
        [
            np.concatenate(
                [r["out"].reshape(PE_E), r["out2"].reshape(DVE_E)]
            )
            for r in res.results
        ]
    )
    return full.astype(np.float32, copy=False), res


def kernel(inputs, centroids):
    full, _ = run(inputs, centroids)
    return full
